# revision 1
# baseline (speedup 1.0000x reference)
"""Trainium2 kernel for nn_LoRALinear (moe_routing).

Math: reference computes out = x @ W.T + einsum('bri,bro->bo', a, b) with
a = A_table[dom].reshape(B,R,IN), b = B_table[dom].reshape(B,R,OUT).
The einsum contracts i over `a` alone, so the LoRA term collapses to a
per-domain table:
    L[d, o] = sum_r (sum_i A_table[d].reshape(R,IN)[r,i]) * B_table[d].reshape(R,OUT)[r,o]
    out = x @ W.T + L[domain_id]

On device this is a single augmented matmul per batch row:
    out[m, :] = [x[m, :], onehot(dom[m])] @ [[W.T], [L]]
with contraction K = 1024 (8 chunks of 128) plus a K=64 one-hot chunk.
The one-hot rows select L rows exactly (0/1 are exact in bf16). The two
K=64 one-hot matmuls per m-tile are packed into disjoint PE row groups
(tile_position) so they run concurrently.

Sharding: data-parallel over batch across 8 cores; the augmented weight is
replicated.

Device layout: the host pre-transposes activations into chunk-major form
xa[p, mb, k, j] = xaT[k*128 + p, mb*MB + j] so each m-block is a single
contiguous-per-partition DMA covering all 9 K-chunks (chunk 8 carries the
one-hot rows duplicated into both half-partitions).
"""

import functools

import numpy as np

import concourse.mybir as mybir
import concourse.tile as tile
from concourse import bacc, bass_utils

B, D, R, ND = 16384, 1024, 8, 64
N_CORES = 8
BS = B // N_CORES            # 2048 batch rows per core
NKW = 8                      # K chunks of 128 for the dense W part
NK = NKW + 1                 # + one-hot chunk
MB = 512                     # batch rows per x block DMA
NMB = BS // MB               # 4 blocks
OH = 512                     # psum free dim (one bank)


@functools.lru_cache(maxsize=1)
def _build():
    nc = bacc.Bacc(None, target_bir_lowering=False, debug=False)
    bf16 = mybir.dt.bfloat16
    xa = nc.dram_tensor("xa", [128, NMB * NK * MB], bf16, kind="ExternalInput")
    wa = nc.dram_tensor("wa", [NKW * 128, D], bf16, kind="ExternalInput")
    # L table packed for row-group concurrency: rows 0:64 = L[:, 0:512],
    # rows 64:128 = L[:, 512:1024]
    w8 = nc.dram_tensor("w8", [128, OH], bf16, kind="ExternalInput")
    out = nc.dram_tensor("out", [BS, D], mybir.dt.float32, kind="ExternalOutput")

    with tile.TileContext(nc) as tc:
        with (
            tc.tile_pool(name="w", bufs=1) as wpool,
            tc.tile_pool(name="x", bufs=2) as xpool,
            tc.tile_pool(name="o", bufs=4) as opool,
            tc.tile_pool(name="ps", bufs=7, space="PSUM") as pspool,
            tc.tile_pool(name="dps", bufs=1, space="PSUM") as dpspool,
        ):
            # Warm the PE (HAM clock gate) with dummy matmuls on a scratch
            # tile while the first DMAs stream in; otherwise the first ~12
            # real matmuls run at half clock.
            scratch = wpool.tile([128, OH], bf16, tag="scratch")
            nc.gpsimd.memset(scratch[:], 0.0)
            dps = dpspool.tile([128, OH], mybir.dt.float32, tag="dps")
            for i in range(12):
                nc.tensor.matmul(
                    dps[:],
                    scratch[:, 0:128],
                    scratch[:],
                    start=(i == 0),
                    stop=(i == 11),
                )

            # x block 0 first so its transfer overlaps the W preload.
            xts = {}
            xt0 = xpool.tile([128, NK * MB], bf16, tag="x")
            nc.sync.dma_start(xt0[:], xa[:, 0 : NK * MB])
            xts[0] = xt0

            wts = []
            for k in range(NKW):
                wt = wpool.tile([128, D], bf16, tag=f"w{k}")
                nc.sync.dma_start(wt[:], wa[k * 128 : (k + 1) * 128, :])
                wts.append(wt)
            w8t = wpool.tile([128, OH], bf16, tag="w8")
            nc.sync.dma_start(w8t[:], w8[:, :])

            def xsl(xt, k, mt):
                return xt[:, k * MB + mt * 128 : k * MB + (mt + 1) * 128]

            def finish(xt, mt, pss, mb):
                """One-hot row-group-packed matmuls + psum copies + out DMA."""
                nc.tensor.matmul(
                    pss[0][:],
                    xt[0:64, NKW * MB + mt * 128 : NKW * MB + (mt + 1) * 128],
                    w8t[0:64, :],
                    start=False,
                    stop=True,
                    tile_position=(0, 0),
                )
                nc.tensor.matmul(
                    pss[1][:],
                    xt[64:128, NKW * MB + mt * 128 : NKW * MB + (mt + 1) * 128],
                    w8t[64:128, :],
                    start=False,
                    stop=True,
                    tile_position=(64, 0),
                )
                ot = opool.tile([128, D], mybir.dt.float32, tag="ot")
                nc.vector.tensor_copy(ot[:, 0:OH], pss[0][:])
                nc.scalar.copy(ot[:, OH : 2 * OH], pss[1][:])
                m0 = mb * MB + mt * 128
                nc.sync.dma_start(out[m0 : m0 + 128, :], ot[:])

            # First two m-tiles: k-interleaved across 4 psum groups so each
            # arriving W chunk immediately feeds 4 matmuls (keeps the PE fed
            # while W streams in).
            pss = {}
            for g in range(4):
                psg = pspool.tile([128, OH], mybir.dt.float32, tag="ps")
                pss[g] = psg
            for k in range(NKW):
                for g in range(4):
                    mt, oh = divmod(g, 2)
                    nc.tensor.matmul(
                        pss[g][:],
                        xsl(xt0, k, mt),
                        wts[k][:, oh * OH : (oh + 1) * OH],
                        start=(k == 0),
                        stop=False,
                    )
            finish(xt0, 0, (pss[0], pss[1]), 0)
            finish(xt0, 1, (pss[2], pss[3]), 0)

            for mb in range(NMB):
                if mb not in xts:
                    xtn = xpool.tile([128, NK * MB], bf16, tag="x")
                    nc.sync.dma_start(
                        xtn[:], xa[:, mb * NK * MB : (mb + 1) * NK * MB]
                    )
                    xts[mb] = xtn
                xt = xts[mb]
                for mt in range(MB // 128):
                    if mb == 0 and mt < 2:
                        continue  # handled by the k-interleaved prologue
                    ps0 = pspool.tile([128, OH], mybir.dt.float32, tag="ps")
                    ps1 = pspool.tile([128, OH], mybir.dt.float32, tag="ps")
                    for k in range(NKW):
                        nc.tensor.matmul(
                            ps0[:],
                            xsl(xt, k, mt),
                            wts[k][:, 0:OH],
                            start=(k == 0),
                            stop=False,
                        )
                    for k in range(NKW):
                        nc.tensor.matmul(
                            ps1[:],
                            xsl(xt, k, mt),
                            wts[k][:, OH : 2 * OH],
                            start=(k == 0),
                            stop=False,
                        )
                    finish(xt, mt, (ps0, ps1), mb)

    nc.compile()
    return nc


def _prepare(x, W, A_table, B_table, domain_id):
    import ml_dtypes

    bf16 = np.dtype(ml_dtypes.bfloat16)
    x = np.asarray(x, dtype=np.float32)
    W = np.asarray(W, dtype=np.float32)
    A = np.asarray(A_table, dtype=np.float64)
    Bt = np.asarray(B_table, dtype=np.float64)
    dom = np.asarray(domain_id).astype(np.int64)

    sA = A.reshape(ND, R, D).sum(axis=2)                        # [ND, R]
    L = np.einsum("dr,dro->do", sA, Bt.reshape(ND, R, D))       # [ND, D]
    Lb = L.astype(np.float32).astype(bf16)

    wa = np.ascontiguousarray(W.T.astype(bf16))                 # [D, D]
    w8 = np.empty((128, OH), dtype=bf16)
    w8[0:ND] = Lb[:, 0:OH]
    w8[ND : 2 * ND] = Lb[:, OH : 2 * OH]

    xT = np.ascontiguousarray(x.T).astype(bf16)                 # [D, B]
    onehotT = (
        np.arange(ND, dtype=np.int64)[:, None] == dom[None, :]
    ).astype(bf16)                                              # [ND, B]

    in_maps = []
    for c in range(N_CORES):
        sl = slice(c * BS, (c + 1) * BS)
        xaT_c = np.empty((NK * 128, BS), dtype=bf16)
        xaT_c[: NKW * 128] = xT[:, sl]
        xaT_c[NKW * 128 : NKW * 128 + ND] = onehotT[:, sl]
        xaT_c[NKW * 128 + ND :] = onehotT[:, sl]                # duplicate
        # chunk-major: xa[p, mb, k, j] = xaT_c[k*128 + p, mb*MB + j]
        xa_c = np.ascontiguousarray(
            xaT_c.reshape(NK, 128, NMB, MB).transpose(1, 2, 0, 3)
        ).reshape(128, NMB * NK * MB)
        in_maps.append({"xa": xa_c, "wa": wa, "w8": w8})
    return in_maps


def kernel(x, W, A_table, B_table, domain_id, _trace=False):
    in_maps = _prepare(x, W, A_table, B_table, domain_id)
    nc = _build()
    res = bass_utils.run_bass_kernel_spmd(
        nc, in_maps, core_ids=list(range(N_CORES)), trace=_trace
    )
    out = np.concatenate([res.results[c]["out"] for c in range(N_CORES)], axis=0)
    if _trace:
        kernel.last_results = res
    return out



# revision 3
# speedup vs baseline: 1.0140x; 1.0140x over previous
"""Trainium2 kernel for nn_LoRALinear (moe_routing).

Math: reference computes out = x @ W.T + einsum('bri,bro->bo', a, b) with
a = A_table[dom].reshape(B,R,IN), b = B_table[dom].reshape(B,R,OUT).
The einsum contracts i over `a` alone, so the LoRA term collapses to a
per-domain table:
    L[d, o] = sum_r (sum_i A_table[d].reshape(R,IN)[r,i]) * B_table[d].reshape(R,OUT)[r,o]
    out = x @ W.T + L[domain_id]

On device this is a single augmented matmul per batch row:
    out[m, :] = [x[m, :], onehot(dom[m])] @ [[W.T], [L]]
with contraction K = 1024 (8 chunks of 128) plus a one-hot chunk. The
one-hot chunk is a regular K=128 chunk: the 64 one-hot rows are duplicated
into both half-partitions, and the L table is placed block-diagonally in a
[128, 1024] tile (L[:, :512] in rows 0:64 of cols 0:512, L[:, 512:] in rows
64:128 of cols 512:1024, zeros elsewhere) so each psum half accumulates its
L contribution exactly once. 0/1 and zeros are exact in bf16.

Sharding: data-parallel over batch across 8 cores; weights replicated.

Schedule: vector-engine memset + a few warmup matmuls release the PE HAM
clock gate during the initial DMA fill; W and x chunks for the first block
are interleaved per-chunk so real matmuls start as early as possible, with
a 6-psum-group prologue (m-tiles 0-2) that consumes each arriving chunk
for longer than the next chunk's DMA takes. Input loads ride the sync-
engine HWDGE ring; output stores ride the scalar-engine ring. Output is
stored as bf16 (host upcasts) to halve store traffic and shorten the tail.

Device layout: the host pre-transposes activations into chunk-major form
xa[p, mb, k, j] = xaT[k*128 + p, mb*MB + j] so each block/chunk is one
contiguous-per-partition DMA.
"""

import functools

import numpy as np

import concourse.mybir as mybir
import concourse.tile as tile
from concourse import bacc, bass_utils

B, D, R, ND = 16384, 1024, 8, 64
N_CORES = 8
BS = B // N_CORES            # 2048 batch rows per core
NKW = 8                      # K chunks of 128 for the dense W part
NK = NKW + 1                 # + one-hot chunk
MB = 512                     # batch rows per x chunk
NMB = BS // MB               # 4 blocks
OH = 512                     # psum free dim (one bank)
NWARM = 4                    # PE warmup matmuls (HAM clock-gate release)
NPRO = 3                     # m-tiles covered by the k-interleaved prologue


@functools.lru_cache(maxsize=1)
def _build():
    nc = bacc.Bacc(None, target_bir_lowering=False, debug=False)
    bf16 = mybir.dt.bfloat16
    f32 = mybir.dt.float32
    xa = nc.dram_tensor("xa", [128, NMB * NK * MB], bf16, kind="ExternalInput")
    wa = nc.dram_tensor("wa", [NKW * 128, D], bf16, kind="ExternalInput")
    # One-hot chunk weights [128, 1024]: rows 0:64 of cols 0:512 hold
    # L[:, 0:512], rows 64:128 of cols 512:1024 hold L[:, 512:1024].
    w8 = nc.dram_tensor("w8", [128, 2 * OH], bf16, kind="ExternalInput")
    out = nc.dram_tensor("out", [BS, D], bf16, kind="ExternalOutput")

    with tile.TileContext(nc) as tc:
        with (
            tc.tile_pool(name="w", bufs=1) as wpool,
            tc.tile_pool(name="x0", bufs=NK) as x0pool,
            tc.tile_pool(name="x", bufs=2) as xpool,
            tc.tile_pool(name="o", bufs=4) as opool,
            tc.tile_pool(name="ps", bufs=7, space="PSUM") as pspool,
            tc.tile_pool(name="dps", bufs=1, space="PSUM") as dpspool,
        ):
            # Warm the PE (HAM clock gate) with dummy matmuls while the
            # first DMAs stream in; memset on the vector engine so warmup
            # isn't gated on slow gpsimd dispatch.
            scratch = wpool.tile([128, OH], bf16, tag="scratch")
            nc.vector.memset(scratch[:], 0.0)
            dps = dpspool.tile([128, OH], f32, tag="dps")
            for i in range(NWARM):
                nc.tensor.matmul(
                    dps[:],
                    scratch[:, 0:128],
                    scratch[:],
                    start=(i == 0),
                    stop=(i == NWARM - 1),
                )

            # Interleave W chunk k with x block-0 chunk k so the first real
            # matmul unblocks after ~390KB instead of ~3.2MB.
            wts, x0 = [], []
            for k in range(NKW):
                wt = wpool.tile([128, D], bf16, tag=f"w{k}")
                nc.sync.dma_start(wt[:], wa[k * 128 : (k + 1) * 128, :])
                wts.append(wt)
                xk = x0pool.tile([128, MB], bf16, tag="x0")
                nc.sync.dma_start(xk[:], xa[:, k * MB : (k + 1) * MB])
                x0.append(xk)
            w8t = wpool.tile([128, 2 * OH], bf16, tag="w8")
            nc.sync.dma_start(w8t[:], w8[:, :])
            wts.append(w8t)
            x8 = x0pool.tile([128, MB], bf16, tag="x0")
            nc.sync.dma_start(x8[:], xa[:, NKW * MB : NK * MB])
            x0.append(x8)

            xts = {0: None}

            def xsl(mb, k, mt):
                if mb == 0:
                    return x0[k][:, mt * 128 : (mt + 1) * 128]
                t = xts[mb]
                return t[:, k * MB + mt * 128 : k * MB + (mt + 1) * 128]

            def rhs(k, half):
                return wts[k][:, half * OH : (half + 1) * OH]

            def store(mb, mt, ot, half):
                m0 = mb * MB + mt * 128
                nc.scalar.dma_start(
                    out[m0 : m0 + 128, half * OH : (half + 1) * OH],
                    ot[:, half * OH : (half + 1) * OH],
                )

            # Prologue: k-interleaved across 6 psum groups (m-tiles 0-2 of
            # block 0) so each arriving W/x chunk feeds 6 matmuls — longer
            # than the next chunk's DMA — keeping the PE fed during fill.
            pss = []
            for g in range(2 * NPRO):
                pss.append(
                    pspool.tile([128, OH], f32, tag="ps", name=f"psp{g}")
                )
            for k in range(NK):
                for g in range(2 * NPRO):
                    mt, half = divmod(g, 2)
                    nc.tensor.matmul(
                        pss[g][:],
                        xsl(0, k, mt),
                        rhs(k, half),
                        start=(k == 0),
                        stop=(k == NK - 1),
                    )
            for mt in range(NPRO):
                ot = opool.tile([128, D], bf16, tag="ot")
                nc.vector.tensor_copy(ot[:, 0:OH], pss[2 * mt][:])
                store(0, mt, ot, 0)
                nc.scalar.copy(ot[:, OH : 2 * OH], pss[2 * mt + 1][:])
                store(0, mt, ot, 1)

            # Main loop: per m-tile, 9 chunks into ps0 (cols 0:512) then 9
            # into ps1; the half-0 copy+store overlaps ps1's matmuls.
            tiles = [(0, mt) for mt in range(NPRO, MB // 128)]
            for mb in range(1, NMB):
                tiles += [(mb, mt) for mt in range(MB // 128)]
            last = tiles[-1]
            for mb, mt in tiles:
                if mb not in xts:
                    xtn = xpool.tile([128, NK * MB], bf16, tag="x")
                    nc.sync.dma_start(
                        xtn[:], xa[:, mb * NK * MB : (mb + 1) * NK * MB]
                    )
                    xts[mb] = xtn
                ps0 = pspool.tile([128, OH], f32, tag="ps")
                ps1 = pspool.tile([128, OH], f32, tag="ps")
                ot = opool.tile([128, D], bf16, tag="ot")
                for k in range(NK):
                    nc.tensor.matmul(
                        ps0[:], xsl(mb, k, mt), rhs(k, 0),
                        start=(k == 0), stop=(k == NK - 1),
                    )
                nc.vector.tensor_copy(ot[:, 0:OH], ps0[:])
                store(mb, mt, ot, 0)
                for k in range(NK):
                    nc.tensor.matmul(
                        ps1[:], xsl(mb, k, mt), rhs(k, 1),
                        start=(k == 0), stop=(k == NK - 1),
                    )
                if (mb, mt) == last:
                    # Split the final copy across both engines to shorten
                    # the serial tail after the last matmul.
                    nc.scalar.copy(ot[:, OH : OH + 256], ps1[:, 0:256])
                    nc.vector.tensor_copy(ot[:, OH + 256 : 2 * OH], ps1[:, 256:OH])
                else:
                    nc.scalar.copy(ot[:, OH : 2 * OH], ps1[:])
                store(mb, mt, ot, 1)

    nc.compile()
    return nc


def _prepare(x, W, A_table, B_table, domain_id):
    import ml_dtypes

    bf16 = np.dtype(ml_dtypes.bfloat16)
    x = np.asarray(x, dtype=np.float32)
    W = np.asarray(W, dtype=np.float32)
    A = np.asarray(A_table, dtype=np.float64)
    Bt = np.asarray(B_table, dtype=np.float64)
    dom = np.asarray(domain_id).astype(np.int64)

    sA = A.reshape(ND, R, D).sum(axis=2)                        # [ND, R]
    L = np.einsum("dr,dro->do", sA, Bt.reshape(ND, R, D))       # [ND, D]
    Lb = L.astype(np.float32).astype(bf16)

    wa = np.ascontiguousarray(W.T.astype(bf16))                 # [D, D]
    w8 = np.zeros((128, 2 * OH), dtype=bf16)
    w8[0:ND, 0:OH] = Lb[:, 0:OH]
    w8[ND : 2 * ND, OH : 2 * OH] = Lb[:, OH : 2 * OH]

    xT = np.ascontiguousarray(x.T).astype(bf16)                 # [D, B]
    onehotT = (
        np.arange(ND, dtype=np.int64)[:, None] == dom[None, :]
    ).astype(bf16)                                              # [ND, B]

    in_maps = []
    for c in range(N_CORES):
        sl = slice(c * BS, (c + 1) * BS)
        xaT_c = np.empty((NK * 128, BS), dtype=bf16)
        xaT_c[: NKW * 128] = xT[:, sl]
        xaT_c[NKW * 128 : NKW * 128 + ND] = onehotT[:, sl]
        xaT_c[NKW * 128 + ND :] = onehotT[:, sl]                # duplicate
        # chunk-major: xa[p, mb, k, j] = xaT_c[k*128 + p, mb*MB + j]
        xa_c = np.ascontiguousarray(
            xaT_c.reshape(NK, 128, NMB, MB).transpose(1, 2, 0, 3)
        ).reshape(128, NMB * NK * MB)
        in_maps.append({"xa": xa_c, "wa": wa, "w8": w8})
    return in_maps


def kernel(x, W, A_table, B_table, domain_id, _trace=False):
    in_maps = _prepare(x, W, A_table, B_table, domain_id)
    nc = _build()
    res = bass_utils.run_bass_kernel_spmd(
        nc, in_maps, core_ids=list(range(N_CORES)), trace=_trace
    )
    out = np.concatenate(
        [res.results[c]["out"] for c in range(N_CORES)], axis=0
    ).astype(np.float32)
    if _trace:
        kernel.last_results = res
    return out
